# revision 1
# baseline (speedup 1.0000x reference)
"""Causal single-head attention (B=4, N=2048, E=1024, D=64) on 8 TRN2 NeuronCores.

Sharding: core i handles batch b = i//2, query rows with parity p = i%2
(rows p, p+2, ...). The row-interleaved split makes the causal workload
identical on every core, so one SPMD program serves all 8. K/V are loaded in
full per core (no collectives); Q is the strided half.

The kernel streams K/V in 8 strips of 256 keys so that projections, scores,
softmax and AV all pipeline behind the DMA stream. Inputs stream in fp16
(halves HBM traffic; inputs are O(1)-scaled so fp16 rounding costs ~5e-4
relative error); on-chip score/AV matmuls run in fp32r at full PE rate with
fp32 PSUM accumulation:
  prologue: qT = Wq.T @ Q.T  [64, 1024]  (4 blocks of 256)
  per strip s (keys [512s, 512s+512)):
    kT_s = Wk.T @ KT_s, vT_s = Wv.T @ VT_s    (PSUM over 8 E-chunks)
    vT_s -> PE-transpose -> v1 rows [512s..] ([k, 65], ones column)
    for q-block j >= s, chunk c in 4s..4s+3:
      e = exp((kT_c.T @ qT_j) / 8) [* causal 0/1 mask if j == s]
      po[j] += v1_c.T @ e          [65, 256] (row 64 = softmax denominator)
    epilogue for block s (po[s] complete):
      out_s = transpose(po[s])[:, :64] * recip(transpose(po[s])[:, 64])
"""
import os
import sys

sys.path.insert(0, "/opt/trn_rl_repo")

import numpy as np

B, N, E, D = 4, 2048, 1024, 64
NQL = N // 2      # local q rows per core
QB = 256          # local q-block width (in qT columns)
KC = 128          # k chunk
EC = 128          # E chunk
NEC = E // EC     # 8
SW = 256          # strip width (keys per strip)
NS = N // SW      # 8 strips
PRECISION = os.environ.get("KERNEL_PRECISION", "f16")

_NC_CACHE = {}


def _build_nc(reps=1):
    from concourse import bacc, mybir, tile
    from concourse.masks import make_identity

    f32 = mybir.dt.float32
    if PRECISION == "f32":
        f32r = mybir.dt.float32
        in_dt = mybir.dt.float32
    elif PRECISION == "f16":
        f32r = mybir.dt.float32r
        in_dt = mybir.dt.float16
    else:
        f32r = mybir.dt.float32r
        in_dt = mybir.dt.float32r
    in_store = mybir.dt.float16 if PRECISION == "f16" else mybir.dt.float32
    av_dt = mybir.dt.float16 if PRECISION == "f16" else f32r
    kq_dt = mybir.dt.float16 if PRECISION == "f16" else f32r
    mask_dt = mybir.dt.float16 if PRECISION == "f16" else mybir.dt.bfloat16
    AF = mybir.ActivationFunctionType

    nc = bacc.Bacc()
    KT = nc.dram_tensor("KT", [NS, EC, NEC, SW], in_store, kind="ExternalInput")
    QT = nc.dram_tensor("QT", [EC, NEC, NQL], in_store, kind="ExternalInput")
    VT = nc.dram_tensor("VT", [NS, EC, NEC, SW], in_store, kind="ExternalInput")
    WK = nc.dram_tensor("WK", [EC, NEC, D], in_store, kind="ExternalInput")
    WQ = nc.dram_tensor("WQ", [EC, NEC, D], in_store, kind="ExternalInput")
    WV = nc.dram_tensor("WV", [EC, NEC, D], in_store, kind="ExternalInput")
    MASK = nc.dram_tensor("MASK", [KC, 4, QB], mask_dt, kind="ExternalInput")
    OUT = nc.dram_tensor("OUT", [NQL // QB, KC, 2, D], f32, kind="ExternalOutput")

    with tile.TileContext(nc) as tc:
        for _rep in range(reps):
            with (
                tc.tile_pool(name=f"consts{_rep}", bufs=1) as consts,
                tc.tile_pool(name=f"qin{_rep}", bufs=2) as qin,
                tc.tile_pool(name=f"kin{_rep}", bufs=2) as kin,
                tc.tile_pool(name=f"vin{_rep}", bufs=2) as vin,
                tc.tile_pool(name=f"proj{_rep}", bufs=1) as proj,
                tc.tile_pool(name=f"expp{_rep}", bufs=6) as expp,
                tc.tile_pool(name=f"epi{_rep}", bufs=2) as epi,
                tc.tile_pool(name=f"psA{_rep}", bufs=1, space="PSUM") as psA,
            ):
                # ---- constants ----
                wk_all = consts.tile([EC, NEC, D], in_dt, tag="wk")
                wq_all = consts.tile([EC, NEC, D], in_dt, tag="wq")
                wv_all = consts.tile([EC, NEC, D], in_dt, tag="wv")
                nc.sync.dma_start(wq_all[:], WQ[:].bitcast(in_dt))
                nc.sync.dma_start(wk_all[:], WK[:].bitcast(in_dt))
                nc.sync.dma_start(wv_all[:], WV[:].bitcast(in_dt))
                masks = consts.tile([KC, 4, QB], mask_dt, tag="mask")
                nc.sync.dma_start(masks[:], MASK[:])
                ident = consts.tile([128, 128], f32, tag="ident")
                make_identity(nc, ident[:])

                kT_sb = proj.tile([D, N], kq_dt, tag="kT")
                qT_sb = proj.tile([D, NQL], kq_dt, tag="qT")
                v1_sb = proj.tile([KC, N // KC, D + 1], av_dt, tag="v1")
                nc.gpsimd.memset(v1_sb[:].bitcast(mybir.dt.float32 if av_dt != mybir.dt.float16 else mybir.dt.float16), 1.0)

                # PSUM pools: po[0..3] (4 banks) + ps/pq2 (2) + pkq (1) + pv/pvt (1)
                po = [psA.tile([D + 1, QB], f32, tag=f"po{j}", name=f"po{j}", bufs=1)
                      for j in range(4)]

                # ---- prologue: q projections (4 blocks of 256) ----
                qt = qin.tile([EC, NEC, NQL], in_dt, tag="qt", bufs=1)
                nc.sync.dma_start(qt[:], QT[:].bitcast(in_dt))
                for j in range(4):
                    pkq = psA.tile([D, SW], f32, tag="pkq", name="pkq", bufs=1)
                    for c in range(NEC):
                        nc.tensor.matmul(pkq[:, 0:QB], wq_all[:, c, :],
                                         qt[:, c, QB * j:QB * (j + 1)],
                                         start=(c == 0), stop=(c == NEC - 1))
                    nc.vector.tensor_copy(qT_sb[:, QB * j:QB * (j + 1)], pkq[:, 0:QB])

                # ---- strips ----
                for s in range(NS):
                    # k projection for keys [SW*s, SW*(s+1))
                    kt = kin.tile([EC, NEC, SW], in_dt, tag="kt")
                    nc.sync.dma_start(kt[:], KT[s].bitcast(in_dt))
                    pkq = psA.tile([D, SW], f32, tag="pkq", name="pkq", bufs=1)
                    for c in range(NEC):
                        nc.tensor.matmul(pkq[:], wk_all[:, c, :], kt[:, c, :],
                                         start=(c == 0), stop=(c == NEC - 1))
                    nc.vector.tensor_copy(kT_sb[:, SW * s:SW * (s + 1)], pkq[:])

                    # v projection + transpose into v1
                    vt = vin.tile([EC, NEC, SW], in_dt, tag="vt")
                    nc.sync.dma_start(vt[:], VT[s].bitcast(in_dt))
                    pv = psA.tile([KC, SW // KC, D], f32, tag="pv", name="pv", bufs=1)
                    for t in range(SW // KC):
                        for c in range(NEC):
                            nc.tensor.matmul(pv[:, t, :], vt[:, c, KC * t:KC * (t + 1)],
                                             wv_all[:, c, :],
                                             start=(c == 0), stop=(c == NEC - 1))
                    for t in range(SW // KC):
                        nc.vector.tensor_copy(v1_sb[:, (SW // KC) * s + t, 0:D], pv[:, t, :])

                    # attention for the new k chunks against q-blocks j >= s//2
                    for j in range((SW * s) // (2 * QB), 4):
                        for m in range(SW // KC):
                            c = (SW // KC) * s + m
                            ps = psA.tile([KC, QB], f32, tag="ps", name="ps", bufs=2)
                            nc.tensor.matmul(ps[:], kT_sb[:, KC * c:KC * (c + 1)],
                                             qT_sb[:, QB * j:QB * (j + 1)], start=True, stop=True)
                            ex = expp.tile([KC, QB], av_dt, tag="ex")
                            nc.scalar.activation(ex[:], ps[:], AF.Exp, scale=0.125)
                            if c >= 4 * j:
                                nc.vector.tensor_mul(ex[:], ex[:], masks[:, c - 4 * j, :])
                            nc.tensor.matmul(po[j][:], v1_sb[:, c, :], ex[:],
                                             start=(c == 0), stop=(c == 4 * j + 3))

                    # epilogue when a q-block just completed (strip covered its last chunks)
                    if (SW * (s + 1)) % (2 * QB) == 0:
                        jj = (SW * (s + 1)) // (2 * QB) - 1
                        pot = epi.tile([D + 1, QB], f32, tag="pot")
                        nc.vector.tensor_copy(pot[:], po[jj][:])
                        ob = epi.tile([KC, 2, D], f32, tag="ob")
                        for h in range(2):
                            pq2 = psA.tile([KC, D + 1], f32, tag="ps", name="pq2", bufs=2)
                            nc.tensor.transpose(pq2[:], pot[:, KC * h:KC * (h + 1)],
                                                ident[0:D + 1, 0:D + 1])
                            rcp = epi.tile([KC, 1], f32, tag="rcp")
                            nc.vector.reciprocal(rcp[:], pq2[:, D:D + 1])
                            nc.vector.tensor_scalar_mul(ob[:, h, :], pq2[:, 0:D], rcp[:])
                        nc.sync.dma_start(OUT[jj], ob[:])

    nc.finalize()
    return nc


def get_nc(reps=1):
    key = ("nc", reps)
    if key not in _NC_CACHE:
        _NC_CACHE[key] = _build_nc(reps)
    return _NC_CACHE[key]


def shard_inputs(K, Q, V, Wk, Wq, Wv):
    in_np = np.float16 if PRECISION == "f16" else np.float32
    K, Q, V = np.asarray(K), np.asarray(Q), np.asarray(V)
    Wkx, Wqx, Wvx = (
        np.ascontiguousarray(np.asarray(a, dtype=np.float32).reshape(NEC, EC, D).transpose(1, 0, 2)).astype(in_np)
        for a in (Wk, Wq, Wv))
    kk = np.arange(KC)
    qq = np.arange(QB)
    masks = {}
    for p in range(2):
        m4 = np.stack([
            (kk[:, None] + KC * m <= 2 * qq[None, :] + p).astype(np.float32)
            for m in range(4)
        ])  # [4, 128, 256]
        if PRECISION == "f16":
            mdt = np.float16
        else:
            import ml_dtypes
            mdt = ml_dtypes.bfloat16
        masks[p] = np.ascontiguousarray(m4.transpose(1, 0, 2).astype(mdt))
    in_maps = []
    for core in range(8):
        b, p = core // 2, core % 2
        kx = np.ascontiguousarray(
            K[b].T.reshape(NEC, EC, NS, SW).transpose(2, 1, 0, 3)).astype(in_np)
        vx = np.ascontiguousarray(
            V[b].T.reshape(NEC, EC, NS, SW).transpose(2, 1, 0, 3)).astype(in_np)
        qx = np.ascontiguousarray(
            Q[b].T[:, p::2].reshape(NEC, EC, NQL).transpose(1, 0, 2)).astype(in_np)
        in_maps.append({
            "KT": kx,
            "QT": qx,
            "VT": vx,
            "WK": Wkx, "WQ": Wqx, "WV": Wvx,
            "MASK": masks[p],
        })
    return in_maps


def gather_outputs(outs):
    full = np.zeros((B, N, D), np.float32)
    for core in range(8):
        b, p = core // 2, core % 2
        o = np.asarray(outs[core])
        if o.ndim == 4:  # [NS, KC, 2, D] -> local rows [NS*2*KC, D]
            o = o.transpose(0, 2, 1, 3).reshape(NQL, D)
        full[b, p::2] = o
    return full


def kernel(K, Q, V, Wk, Wq, Wv):
    from concourse.bass_utils import run_bass_kernel_spmd

    in_maps = shard_inputs(K, Q, V, Wk, Wq, Wv)
    nc = get_nc()
    res = run_bass_kernel_spmd(nc, in_maps, list(range(8)))
    return gather_outputs([res.results[i]["OUT"] for i in range(8)])



# revision 28
# speedup vs baseline: 1.5234x; 1.5234x over previous
"""Causal single-head attention (B=4, N=2048, E=1024, D=64) on 8 TRN2 NeuronCores.

Sharding: core i handles batch b = i//2, query rows with parity p = i%2
(rows p, p+2, ...). The row-interleaved split makes the causal workload
identical on every core, so one SPMD program serves all 8. K/V are loaded in
full per core (no collectives); Q is the strided half.

DMA-minimizing precision scheme (tolerance 2e-2; measured end-to-end err
~7e-3 in numpy):
  - Raw K, Q, V stream in fp8 e3m4 (1 B/elem); V strip 0 (keys 0..255) in
    fp16 so the near-diagonal early rows keep full V precision.
  - Weights stay fp16 (tiny DMA); projections are mixed-dtype matmuls
    (e3m4 activations x fp16 weights). Set KERNEL_MIXED_W=0 to fall back to
    e3m4 weights scaled x64 (exp scale and host divide compensate).
  - On-chip kT/qT are fp16 (scores in fp16 at full PE rate).
  - ex (exp of scores) and v1 are fp8 e4m3 for q-blocks j>=1, enabling
    MatmulPerfMode.DoubleRow on the AV matmuls (0.5 cyc/row); q-block j=0
    (rows < 512, few-key softmax) uses an fp16 AV path.

Compute layout per core:
  prologue: qT = Wq.T @ Q.T  [64, 1024] fp16 (2 halves of 512)
  per strip s (keys [256s, 256s+256), chunks 2s, 2s+1):
    K/V projected tokens-major into one psum tile [128, 4, 64] (k t0,k t1,
    v t0,v t1), copied to fp16, K halves PE-transposed into kT [64, N];
    v rows copied into v1_8 [128, 16, 65] e4m3 (+ v1_16 for strips 0,1).
    for q-block j >= s//2:  (diag group when j == s//2; odd-s diag groups
    only compute the valid right half, cols 128:256)
      ps[128, 2, W] = scores; ex = exp(ps/8); diag groups multiply the
      0/1 causal mask; AV accumulates into po[j] [65, 256] (row 64 = ones
      column = softmax denominator), DoubleRow for j>=1.
    epilogue when po[j] completes: copy to fp32 sbuf, DMA out; the final
    division num/den happens on host in gather_outputs.
"""
import os
import sys

sys.path.insert(0, "/opt/trn_rl_repo")

import numpy as np

B, N, E, D = 4, 2048, 1024, 64
NQL = N // 2      # local q rows per core
QB = 256          # local q-block width (in qT columns)
KC = 128          # k chunk
EC = 128          # E chunk
NEC = E // EC     # 8
SW = 256          # strip width (keys per strip)
NS = N // SW      # 8 strips
MIXED_W = os.environ.get("KERNEL_MIXED_W", "1") == "1"

_NC_CACHE = {}


def _build_nc(reps=1):
    from concourse import bacc, mybir, tile
    from concourse.masks import make_identity

    f32 = mybir.dt.float32
    f16 = mybir.dt.float16
    f8in = mybir.dt.float8e3
    f8av = mybir.dt.float8e4
    w_dt = f16 if MIXED_W else f8in
    AF = mybir.ActivationFunctionType
    DR = mybir.MatmulPerfMode.DoubleRow

    nc = bacc.Bacc()
    KT0 = nc.dram_tensor("KT0", [EC, NEC, SW], f8in, kind="ExternalInput")
    KVT = nc.dram_tensor("KVT", [NS - 1, EC, 2, NEC, SW], f8in, kind="ExternalInput")
    QT = nc.dram_tensor("QT", [4, EC, NEC, QB], f8in, kind="ExternalInput")
    VT16 = nc.dram_tensor("VT16", [EC, NEC, SW], f16, kind="ExternalInput")
    WKV = nc.dram_tensor("WKV", [EC, 2, NEC, D], w_dt, kind="ExternalInput")
    WQ = nc.dram_tensor("WQ", [EC, NEC, D], w_dt, kind="ExternalInput")
    WKV8 = nc.dram_tensor("WKV8", [EC, 2, NEC, D], f8av, kind="ExternalInput")
    WQ8 = nc.dram_tensor("WQ8", [EC, NEC, D], f8av, kind="ExternalInput")
    MASK = nc.dram_tensor("MASK", [KC, 4, QB], f8in, kind="ExternalInput")
    OUT = nc.dram_tensor("OUT", [NQL // QB, D + 1, QB], f16, kind="ExternalOutput")

    exp_scale = 0.125 if MIXED_W else 0.125 / 4096.0

    with tile.TileContext(nc) as tc:
        for _rep in range(reps):
            with (
                tc.tile_pool(name=f"consts{_rep}", bufs=1) as consts,
                tc.tile_pool(name=f"qin{_rep}", bufs=1) as qin,
                tc.tile_pool(name=f"kin{_rep}", bufs=3) as kin,
                tc.tile_pool(name=f"vin{_rep}", bufs=2) as vin,
                tc.tile_pool(name=f"proj{_rep}", bufs=1) as proj,
                tc.tile_pool(name=f"kv{_rep}", bufs=2) as kvp,
                tc.tile_pool(name=f"expp{_rep}", bufs=4) as expp,
                tc.tile_pool(name=f"epi{_rep}", bufs=2) as epi,
                tc.tile_pool(name=f"psA{_rep}", bufs=1, space="PSUM") as psA,
            ):
                # ---- constants; DMA order: wkv, k0, v0, wq, qt, masks ----
                wkv_all = consts.tile([EC, 2, NEC, D], w_dt, tag="wkv")
                wq_all = consts.tile([EC, NEC, D], w_dt, tag="wq")
                nc.sync.dma_start(wkv_all[:], WKV[:])
                wk_all = wkv_all[:, 0, :, :]
                wv_all = wkv_all[:, 1, :, :]
                kt0 = kin.tile([EC, NEC, SW], f8in, tag="kt", bufs=1)
                nc.sync.dma_start(kt0[:], KT0[:])
                vt0 = vin.tile([EC, NEC, SW], f16, tag="vt16", bufs=1)
                nc.sync.dma_start(vt0[:], VT16[:])
                nc.sync.dma_start(wq_all[:], WQ[:])
                ident = consts.tile([128, 128], f16, tag="ident")
                make_identity(nc, ident[:])

                kT_sb = proj.tile([D, NS, 2, KC], f16, tag="kT")
                qT_sb = proj.tile([D, NQL], f16, tag="qT")
                v1_16 = proj.tile([KC, 4, D + 1], f16, tag="v116")
                v1_8 = proj.tile([KC, N // KC, 128], f8av, tag="v18")
                nc.gpsimd.memset(v1_16[:], 1.0)
                nc.gpsimd.memset(v1_8[:], 1.0)

                # po padded to 128 partitions: DoubleRow needs col_grp 0xf
                po = [psA.tile([128, QB], f32, tag=f"po{j}", name=f"po{j}", bufs=1)
                      for j in range(4)]

                def poap(j, a=0, b=QB):
                    return po[j][0:D + 1, a:b]

                def poap_full(j, a=0, b=QB):
                    return po[j][:, a:b]

                # ---- prologue: qt block-0 DMA + masks + block-0 projection;
                # blocks 1-3 stream in later and are projected just-in-time ----
                qt = qin.tile([EC, 4, NEC, QB], f8in, tag="qt", bufs=1)
                nc.sync.dma_start(qt[:, 0, :, :], QT[0])
                masks = consts.tile([KC, 4, QB], f8in, tag="mask")
                nc.sync.dma_start(masks[:], MASK[:])
                kvt1 = kin.tile([EC, 2, NEC, SW], f8in, tag="kvt")
                nc.sync.dma_start(kvt1[:], KVT[0])
                wkv8_all = consts.tile([EC, 2, NEC, D], f8av, tag="wkv8")
                wq8_all = consts.tile([EC, NEC, D], f8av, tag="wq8")
                nc.sync.dma_start(wkv8_all[:], WKV8[:])
                nc.sync.dma_start(wq8_all[:], WQ8[:])
                nc.sync.dma_start(qt[:, 1, :, :], QT[1])
                # block 0: e3m4 x fp16-W, D-major
                pq = psA.tile([KC, 2, QB], f32, tag="ps", name="ps", bufs=2)
                for c in range(NEC):
                    nc.tensor.matmul(pq[0:D, 0, :], wq_all[:, c, :],
                                     qt[:, 0, c, :],
                                     start=(c == 0), stop=(c == NEC - 1))
                nc.vector.tensor_copy(qT_sb[:, 0:QB], pq[0:D, 0, :])

                qt8 = qt[:].bitcast(f8av)

                def emit_qblk(h):
                    # e4m3 DoubleRow, tokens-major + PE transpose, x1/16 fixup
                    pq = psA.tile([KC, 2, QB], f32, tag="ps", name="ps", bufs=2)
                    for t in range(2):
                        for c in range(4):
                            nc.tensor.matmul(
                                pq[:, t, 0:D],
                                qt8[:, h, 2 * c:2 * c + 2, KC * t:KC * (t + 1)],
                                wq8_all[:, 2 * c:2 * c + 2, :],
                                start=(c == 0), stop=(c == 3), perf_mode=DR)
                    q16 = kvp.tile([KC, 4, D], f16, tag="kv16")
                    nc.vector.tensor_copy(q16[:, 0:2, :], pq[:, :, 0:D])
                    pkvq = psA.tile([KC, 6, D], f32, tag="pkv", name="pkv", bufs=2)
                    pqT = pkvq[0:D, 4:6, :].bitcast(f16)
                    for t in range(2):
                        nc.tensor.transpose(pqT[:, t, :], q16[:, t, :], ident[:])
                    nc.vector.tensor_scalar_mul(qT_sb[:, QB * h:QB * (h + 1)],
                                                pqT[:], 1.0 / 16.0)

                def emit_av(s, j, ex, is_diag, half, W):
                    if j == 0:
                        # fp16 path, per-chunk matmuls; v1_16 chunks 2s+m
                        for m in range(2):
                            c = 2 * s + m
                            if c == 0:
                                nc.tensor.matmul(poap(0), v1_16[:, 0, :],
                                                 ex[:, 0, :], start=True, stop=False)
                            elif c == 1:
                                nc.tensor.matmul(poap(0, 0, 128), v1_16[:, 1, :],
                                                 ex[:, 1, 0:128], start=False, stop=False)
                                nc.tensor.matmul(poap(0, 128, QB), v1_16[:, 1, :],
                                                 ex[:, 1, 128:QB], start=False, stop=False)
                            else:  # c in (2, 3): right half only
                                nc.tensor.matmul(poap(0, 128, QB), v1_16[:, c, :],
                                                 ex[:, m, 0:W], start=False,
                                                 stop=(c == 3))
                    else:
                        v1s = v1_8[:, 2 * s:2 * s + 2, :]
                        if not is_diag:
                            nc.tensor.matmul(poap_full(j), v1s, ex[:],
                                             start=(s == 0), stop=False,
                                             perf_mode=DR)
                        elif not half:
                            nc.tensor.matmul(poap_full(j, 0, 128), v1s, ex[:, :, 0:128],
                                             start=False, stop=False, perf_mode=DR)
                            nc.tensor.matmul(poap_full(j, 128, QB), v1s, ex[:, :, 128:QB],
                                             start=False, stop=False, perf_mode=DR)
                        else:
                            nc.tensor.matmul(poap_full(j, 128, QB), v1s, ex[:, :, 0:W],
                                             start=False, stop=True, perf_mode=DR)
                    # epilogue when q-block j just completed (odd-s diag group)
                    if is_diag and half:
                        pof = epi.tile([D + 1, QB], f16, tag="pof")
                        nc.vector.tensor_copy(pof[:], poap(j))
                        nc.sync.dma_start(OUT[j], pof[:])


                # ---- strips: emission follows data-arrival order.
                # Group (s, j) is emitted at strip t = max(s, j+1) (qt block j
                # arrives around strip j+1); its AV is deferred to t+1. ----
                pending_avs = []
                for t in range(NS):
                    # strip-t K/V DMA + projections
                    if t == 0:
                        kt, vt = kt0, vt0
                    elif t == 1:
                        kt = kvt1[:, 0, :, :]
                        vt = kvt1[:, 1, :, :]
                    else:
                        kvt = kin.tile([EC, 2, NEC, SW], f8in, tag="kvt")
                        nc.sync.dma_start(kvt[:], KVT[t - 1])
                        kt = kvt[:, 0, :, :].bitcast(f8av)
                        vt = kvt[:, 1, :, :].bitcast(f8av)
                    # qt block DMAs, staggered: block t+1 requested at strip t
                    if 1 <= t <= 2:
                        nc.sync.dma_start(qt[:, t + 1, :, :], QT[t + 1])
                    pkv = psA.tile([KC, 6, D], f32, tag="pkv", name="pkv", bufs=2)
                    if t < 2:
                        for u in range(2):
                            for c in range(NEC):
                                nc.tensor.matmul(pkv[:, u, :], kt[:, c, KC * u:KC * (u + 1)],
                                                 wk_all[:, c, :],
                                                 start=(c == 0), stop=(c == NEC - 1))
                        for u in range(2):
                            for c in range(NEC):
                                nc.tensor.matmul(pkv[:, 2 + u, :], vt[:, c, KC * u:KC * (u + 1)],
                                                 wv_all[:, c, :],
                                                 start=(c == 0), stop=(c == NEC - 1))
                    else:
                        for u in range(2):
                            for c in range(4):
                                nc.tensor.matmul(pkv[:, u, :],
                                                 kt[:, 2 * c:2 * c + 2, KC * u:KC * (u + 1)],
                                                 wkv8_all[:, 0, 2 * c:2 * c + 2, :],
                                                 start=(c == 0), stop=(c == 3), perf_mode=DR)
                        for u in range(2):
                            for c in range(4):
                                nc.tensor.matmul(pkv[:, 2 + u, :],
                                                 vt[:, 2 * c:2 * c + 2, KC * u:KC * (u + 1)],
                                                 wkv8_all[:, 1, 2 * c:2 * c + 2, :],
                                                 start=(c == 0), stop=(c == 3), perf_mode=DR)
                    kv16 = kvp.tile([KC, 4, D], f16, tag="kv16")
                    nc.vector.tensor_copy(kv16[:], pkv[:, 0:4, :])
                    # kT via PE transpose; transpose psum in upper pkv bank
                    pkT = pkv[0:D, 4:6, :].bitcast(f16)
                    for u in range(2):
                        nc.tensor.transpose(pkT[:, u, :], kv16[:, u, :], ident[:])
                    if t < 2:
                        nc.vector.tensor_copy(kT_sb[:, t, :, :], pkT[:])
                        nc.gpsimd.tensor_copy(v1_8[:, 2 * t:2 * t + 2, 0:D], kv16[:, 2:4, :])
                        nc.gpsimd.tensor_copy(v1_16[:, 2 * t:2 * t + 2, 0:D], kv16[:, 2:4, :])
                    else:
                        nc.vector.tensor_scalar_mul(kT_sb[:, t, :, :], pkT[:], 1.0 / 16.0)
                        nc.gpsimd.tensor_scalar_mul(v1_8[:, 2 * t:2 * t + 2, 0:D],
                                                    kv16[:, 2:4, :], 1.0 / 16.0)

                    # just-in-time Q block projection (qT block t, needed by
                    # groups (*, t) emitted at strip t+1)
                    if 1 <= t <= 3:
                        emit_qblk(t)

                    # AVs of groups emitted last strip
                    for av in pending_avs:
                        av()
                    pending_avs = []

                    # groups (s, j) with max(s, j+1) == t, ordered by s
                    for s in range(NS):
                        for j in range(s // 2, 4):
                            if max(s, j + 1) != t:
                                continue
                            is_diag = (j == s // 2)
                            half = is_diag and (s % 2 == 1)
                            c0 = 128 if half else 0
                            W = QB - c0
                            ps = psA.tile([KC, 2, QB], f32, tag="ps", name="ps", bufs=2)
                            for m in range(2):
                                nc.tensor.matmul(ps[:, m, c0:QB],
                                                 kT_sb[:, s, m, :],
                                                 qT_sb[:, QB * j + c0:QB * (j + 1)],
                                                 start=True, stop=True)
                            ex_dt = f16 if j == 0 else f8av
                            ex = expp.tile([KC, 2, QB], ex_dt,
                                           tag=("ex16" if j == 0 else "ex8"), bufs=10)
                            nc.scalar.activation(ex[:, :, 0:W], ps[:, :, c0:QB],
                                                 AF.Exp, scale=exp_scale)
                            if is_diag:
                                nc.vector.tensor_mul(ex[:, :, 0:W], ex[:, :, 0:W],
                                                     masks[:, 2 * (s % 2):2 * (s % 2) + 2, c0:QB])
                            pending_avs.append(
                                lambda s=s, j=j, ex=ex, is_diag=is_diag, half=half, W=W:
                                emit_av(s, j, ex, is_diag, half, W))

                # flush the last strip's groups
                for av in pending_avs:
                    av()
                pending_avs = []

    nc.finalize()
    return nc


def get_nc(reps=1):
    key = ("nc", reps)
    if key not in _NC_CACHE:
        _NC_CACHE[key] = _build_nc(reps)
    return _NC_CACHE[key]


def shard_inputs(K, Q, V, Wk, Wq, Wv):
    import ml_dtypes
    e3 = ml_dtypes.float8_e3m4
    e4 = ml_dtypes.float8_e4m3
    K, Q, V = (np.asarray(a, np.float32) for a in (K, Q, V))
    wks, wqs, wvs = (np.asarray(a, np.float32) for a in (Wk, Wq, Wv))
    Wkx, Wqx, Wvx = (
        np.ascontiguousarray(a.reshape(NEC, EC, D).transpose(1, 0, 2)).astype(np.float16)
        for a in (wks, wqs, wvs))
    Wkv = np.ascontiguousarray(np.stack([Wkx, Wvx], axis=1))  # [EC, 2, NEC, D]
    Wkx8, Wqx8, Wvx8 = (
        np.ascontiguousarray((a * 16).reshape(NEC, EC, D).transpose(1, 0, 2)).astype(e4)
        for a in (wks, wqs, wvs))
    Wkv8 = np.ascontiguousarray(np.stack([Wkx8, Wvx8], axis=1))
    kk = np.arange(KC)
    qq = np.arange(QB)
    masks = {}
    for p in range(2):
        m4 = np.stack([
            (kk[:, None] + KC * m <= 2 * qq[None, :] + p).astype(np.float32)
            for m in range(4)
        ])  # [4, 128, 256]
        masks[p] = np.ascontiguousarray(m4.transpose(1, 0, 2).astype(e3))
    in_maps = []
    for core in range(8):
        b, p = core // 2, core % 2
        kx = np.clip(K[b], -15, 15).T.reshape(NEC, EC, NS, SW).transpose(2, 1, 0, 3)
        vfull = np.clip(V[b], -15, 15).T.reshape(NEC, EC, NS, SW).transpose(2, 1, 0, 3)
        # KVT: strips 1..7, K/V interleaved on dim 2 -> [7, EC, 2, NEC, SW].
        # Strip 1 is e3m4; strips 2-7 carry e4m3 bit patterns (read via
        # bitcast on-chip for the DoubleRow projections).
        kvf = np.stack([kx[1:], vfull[1:]], axis=2)
        kv = np.ascontiguousarray(kvf).astype(e3)
        kv[1:] = kvf[1:].astype(e4).view(np.uint8).view(e3)
        kx0 = np.ascontiguousarray(kx[0]).astype(e3)
        v16 = np.ascontiguousarray(
            V[b].T.reshape(NEC, EC, NS, SW).transpose(2, 1, 0, 3)[0]).astype(np.float16)
        # QT block-major [4, EC, NEC, QB]: block 0 e3m4; blocks 1-3 e4m3
        qf = np.clip(Q[b], -15, 15).T[:, p::2].reshape(NEC, EC, 4, QB).transpose(2, 1, 0, 3)
        qx = np.ascontiguousarray(qf).astype(e3)
        qx[1:] = qf[1:].astype(e4).view(np.uint8).view(e3)
        in_maps.append({
            "KT0": kx0,
            "KVT": kv,
            "QT": qx,
            "VT16": v16,
            "WKV": Wkv, "WQ": Wqx,
            "WKV8": Wkv8, "WQ8": Wqx8,
            "MASK": masks[p],
        })
    return in_maps


def gather_outputs(outs):
    full = np.zeros((B, N, D), np.float32)
    scale = 1.0 if MIXED_W else 64.0
    for core in range(8):
        b, p = core // 2, core % 2
        o = np.asarray(outs[core], np.float32)  # [4, 65, 256]
        num = o[:, 0:D, :] / scale              # [4, 64, 256]
        den = o[:, D, :]                        # [4, 256]
        loc = (num / den[:, None, :]).transpose(0, 2, 1).reshape(NQL, D)
        full[b, p::2] = loc
    return full


def kernel(K, Q, V, Wk, Wq, Wv):
    from concourse.bass_utils import run_bass_kernel_spmd

    in_maps = shard_inputs(K, Q, V, Wk, Wq, Wv)
    nc = get_nc()
    res = run_bass_kernel_spmd(nc, in_maps, list(range(8)))
    return gather_outputs([res.results[i]["OUT"] for i in range(8)])
